# revision 17
# baseline (speedup 1.0000x reference)
"""Trainium2 Bass kernel for multi-head self-attention (B=8, N=1024, C=768, H=12).

Sharding: data-parallel over batch -- one batch element per NeuronCore (8 cores).
Each core computes the full attention for its batch element; no collectives.

Per-core dataflow:
  x [N,C] --PE transpose (f32r single-pass)--> xT [C,N]
  qkvT [3C,N] = w_qkvT.T @ xT            (f32r matmuls, stationary = w chunks)
  per head h:  ST[m,n] = k_h @ q_h^T     (f32r, K=64)
               ET = exp(0.125*ST) -> bf16 (one ACT op per [128,1024] PSUM tile;
                                          no max-subtraction: |scores| < ~3)
               OT_unnorm[d,n], den[n] = [v_h | 1].T @ ET   (bf16 PV, ones-column)
               OT[d,n] = OT_unnorm * bcast(den)^-1  (PE K=1 bcast + fast recip)
  proj is folded incrementally: after each head pair,
               acc[n, :] += OT_hp.T @ w_projT chunk      (acc init = b_proj bcast)
  out = acc

Heads are processed in pairs with both heads' ST/exp emitted before either
head's PV, and the per-head normalization broadcasts batched after both PV
accumulations, so the PE stream stays dense (HAM stays at full clock).
"""

import numpy as np

import concourse.bass as bass
import concourse.tile as tile
from concourse import bacc
from concourse import mybir
from concourse.bass_utils import run_bass_kernel_spmd
from concourse.masks import make_identity

N = 1024
C = 768
H = 12
D = 64
O3 = 3 * C  # 2304
NCORES = 8
SCALE = D**-0.5

F32 = mybir.dt.float32
F32R = mybir.dt.float32r
BF16 = mybir.dt.bfloat16
EXP = mybir.ActivationFunctionType.Exp

NT_N = N // 128  # 8 n-blocks
NT_C = C // 128  # 6 c-chunks


def build_bass():
    nc = bacc.Bacc("TRN2", target_bir_lowering=False, debug=False, num_devices=NCORES)

    x = nc.dram_tensor("x", [N, C], F32R, kind="ExternalInput").ap()
    w_qkv = nc.dram_tensor("w_qkv", [O3, C], F32R, kind="ExternalInput").ap()
    w_proj = nc.dram_tensor("w_proj", [C, C], F32R, kind="ExternalInput").ap()
    b_proj = nc.dram_tensor("b_proj", [1, C], F32R, kind="ExternalInput").ap()
    out = nc.dram_tensor("out", [N, C], F32, kind="ExternalOutput").ap()

    with tile.TileContext(nc) as tc:
        with (
            tc.tile_pool(name="singles", bufs=1) as singles,
            tc.tile_pool(name="xT", bufs=1) as p_xT,
            tc.tile_pool(name="ld", bufs=4) as p_ld,      # x/w natural staging
            tc.tile_pool(name="wT", bufs=2) as p_wT,
            tc.tile_pool(name="wpT", bufs=1) as p_wpT,
            tc.tile_pool(name="qkvT", bufs=2) as p_qkvT,
            tc.tile_pool(name="et", bufs=18) as p_et,
            tc.tile_pool(name="vn", bufs=1) as p_vn,
            tc.tile_pool(name="OT", bufs=3) as p_OT,
            tc.tile_pool(name="acc", bufs=NT_N) as p_acc,
            tc.tile_pool(name="otmp", bufs=2) as p_otmp,
            tc.tile_pool(name="dn", bufs=4) as p_dn,
            tc.tile_pool(name="rcb", bufs=2) as p_rcb,
            # PSUM: 8 banks total
            tc.tile_pool(name="pp_s", bufs=2, space="PSUM") as pp_s,   # 2x2 banks
            tc.tile_pool(name="pp_sm", bufs=4, space="PSUM") as pp_sm,  # 4x1 bank
        ):
            # ---- setup ----
            identity = singles.tile([128, 128], F32, tag="identity")
            make_identity(nc, identity[:])
            identity_r = singles.tile([128, 128], F32R, tag="identity_r")
            nc.vector.tensor_copy(identity_r[:], identity[:])
            ones_f = singles.tile([128, 128], F32, tag="ones_f")
            nc.vector.memset(ones_f[:], 1.0)
            ones = singles.tile([128, 128], F32R, tag="ones")
            nc.vector.tensor_copy(ones[:], ones_f[:])
            ones_b = singles.tile([128, 1], BF16, tag="ones_b")
            nc.vector.tensor_copy(ones_b[:], ones_f[:, 0:1])
            b_row = singles.tile([1, C], F32R, tag="b_row")
            nc.sync.dma_start(b_row[:], b_proj)

            # persistent vn tile: 16 slots of [v_h block (64) | ones] = 65 cols
            vn = p_vn.tile([128, 16 * 65], BF16, tag="vn")
            ones_cols = bass.AP(
                tensor=vn.tensor, offset=vn.offset + 64, ap=[vn.ap[0], [65, 16], [1, 1]]
            )
            ones_rep = bass.AP(
                tensor=ones_b.tensor,
                offset=ones_b.offset,
                ap=[ones_b.ap[0], [0, 16], [1, 1]],
            )
            nc.vector.tensor_copy(ones_cols, ones_rep)

            # ---- output accumulators, initialized to broadcast bias ----
            acc = [
                p_acc.tile([128, C], F32, tag="acc", name=f"acc{_}")
                for _ in range(NT_N)
            ]
            ps_b = pp_s.tile([128, 1024], F32, tag="pp_s")
            nc.tensor.matmul(
                ps_b[:, 0:512], ones[0:1, 0:128], b_row[:, 0:512], start=True, stop=True
            )
            nc.tensor.matmul(
                ps_b[:, 512:768],
                ones[0:1, 0:128],
                b_row[:, 512:768],
                start=True,
                stop=True,
            )
            for i in range(NT_N):
                nc.vector.tensor_copy(acc[i][:], ps_b[:, 0:C])

            # ---- phase X: x -> xT (single [128, 6*1024] f32r tile) ----
            xT = p_xT.tile([128, NT_C * N], F32R, tag="xT")

            for i in range(NT_N):
                xn = p_ld.tile([128, C], F32R, tag="ld")
                nc.sync.dma_start(xn[:], x[i * 128 : (i + 1) * 128, :])
                for j0 in range(0, NT_C, 2):
                    pt = pp_sm.tile([128, 256], F32R, tag="pp_sm")
                    nc.tensor.transpose(
                        pt[:, 0:128], xn[:, j0 * 128 : (j0 + 1) * 128], identity_r[:]
                    )
                    nc.tensor.transpose(
                        pt[:, 128:256],
                        xn[:, (j0 + 1) * 128 : (j0 + 2) * 128],
                        identity_r[:],
                    )
                    dst = bass.AP(
                        tensor=xT.tensor,
                        offset=xT.offset + j0 * N + i * 128,
                        ap=[xT.ap[0], [N, 2], [1, 128]],
                    )
                    nc.vector.tensor_copy(
                        dst, pt[:].rearrange("p (two c) -> p two c", two=2)
                    )

            # ---- wproj -> wpT early (single [128, 6*768] tile) ----
            wpT = p_wpT.tile([128, NT_C * C], F32R, tag="wpT")
            for i in range(NT_C):
                wpn = p_ld.tile([128, C], F32R, tag="ld")
                nc.sync.dma_start(wpn[:], w_proj[i * 128 : (i + 1) * 128, :])
                for j0 in range(0, NT_C, 2):
                    pt = pp_sm.tile([128, 256], F32R, tag="pp_sm")
                    nc.tensor.transpose(
                        pt[:, 0:128], wpn[:, j0 * 128 : (j0 + 1) * 128], identity_r[:]
                    )
                    nc.tensor.transpose(
                        pt[:, 128:256],
                        wpn[:, (j0 + 1) * 128 : (j0 + 2) * 128],
                        identity_r[:],
                    )
                    dst = bass.AP(
                        tensor=wpT.tensor,
                        offset=wpT.offset + j0 * C + i * 128,
                        ap=[wpT.ap[0], [C, 2], [1, 128]],
                    )
                    nc.vector.tensor_copy(
                        dst, pt[:].rearrange("p (two c) -> p two c", two=2)
                    )

            # ---- phase A+B interleaved over head pairs ----
            for hp in range(H // 2):
                # phase A(hp): wT [c, 3*128] chunks then qkvT [128, 3*1024]
                wT = p_wT.tile([128, NT_C * 384], F32R, tag="wT")
                for part in range(3):
                    row0 = part * C + hp * 128
                    wn = p_ld.tile([128, C], F32R, tag="ld")
                    nc.sync.dma_start(wn[:], w_qkv[row0 : row0 + 128, :])
                    for j0 in range(0, NT_C, 2):
                        pt = pp_sm.tile([128, 256], F32R, tag="pp_sm")
                        nc.tensor.transpose(
                            pt[:, 0:128],
                            wn[:, j0 * 128 : (j0 + 1) * 128],
                            identity_r[:],
                        )
                        nc.tensor.transpose(
                            pt[:, 128:256],
                            wn[:, (j0 + 1) * 128 : (j0 + 2) * 128],
                            identity_r[:],
                        )
                        dst = bass.AP(
                            tensor=wT.tensor,
                            offset=wT.offset + j0 * 384 + part * 128,
                            ap=[wT.ap[0], [384, 2], [1, 128]],
                        )
                        nc.vector.tensor_copy(
                            dst, pt[:].rearrange("p (two c) -> p two c", two=2)
                        )

                blk = p_qkvT.tile([128, 3 * N], F32R, tag="qkvT")
                for part in range(3):
                    ps = pp_s.tile([128, 1024], F32, tag="pp_s")
                    for nj in range(2):
                        nsl = slice(nj * 512, (nj + 1) * 512)
                        for j in range(NT_C):
                            nc.tensor.matmul(
                                ps[:, nsl],
                                wT[:, j * 384 + part * 128 : j * 384 + (part + 1) * 128],
                                xT[:, j * N + nj * 512 : j * N + nj * 512 + 512],
                                start=(j == 0),
                                stop=(j == NT_C - 1),
                            )
                    nc.vector.tensor_copy(blk[:, part * N : (part + 1) * N], ps[:])

                # phase B: both heads' ST/exp first, then both heads' PV
                et = {}
                for h2 in range(2):
                    rsl = slice(h2 * 64, h2 * 64 + 64)
                    isl = slice(h2 * 64, h2 * 64 + 64)
                    qT = blk[rsl, 0:N]
                    kT = blk[rsl, N : 2 * N]
                    vT = blk[rsl, 2 * N : 3 * N]

                    # v natural blocks into persistent vn tile (bf16)
                    for t0 in range(0, NT_N, 2):
                        pt = pp_sm.tile([128, 128], F32R, tag="pp_sm")
                        nc.tensor.transpose(
                            pt[:, 0:64],
                            vT[:, t0 * 128 : (t0 + 1) * 128],
                            identity_r[isl, isl],
                        )
                        nc.tensor.transpose(
                            pt[:, 64:128],
                            vT[:, (t0 + 1) * 128 : (t0 + 2) * 128],
                            identity_r[isl, isl],
                        )
                        dst = bass.AP(
                            tensor=vn.tensor,
                            offset=vn.offset + (h2 * 8 + t0) * 65,
                            ap=[vn.ap[0], [65, 2], [1, 64]],
                        )
                        nc.vector.tensor_copy(
                            dst, pt[:].rearrange("p (two c) -> p two c", two=2)
                        )

                    # ST + exp -> ET (bf16)
                    ets = []
                    for t in range(NT_N):
                        ps = pp_s.tile([128, 1024], F32, tag="pp_s")
                        for nj in range(2):
                            nsl = slice(nj * 512, (nj + 1) * 512)
                            nc.tensor.matmul(
                                ps[:, nsl],
                                kT[:, t * 128 : (t + 1) * 128],
                                qT[:, nsl],
                                start=True,
                                stop=True,
                            )
                        e = p_et.tile([128, N], BF16, tag="et")
                        nc.scalar.activation(e[:], ps[:], EXP, scale=SCALE)
                        ets.append(e)
                    et[h2] = ets

                # OT for this pair: per head, both PV accumulations then both
                # normalization broadcasts (peak 4 pp_sm slots, no deadlock)
                OT_hp = p_OT.tile([128, N], F32R, tag="OT")
                ot_dst = p_otmp.tile([64, N], F32R, tag="otmp")
                for h2 in range(2):
                    ets = et[h2]
                    po = []
                    dns = []
                    for nj in range(2):
                        nsl = slice(nj * 512, (nj + 1) * 512)
                        p_ = pp_sm.tile([65, 512], F32, tag="pp_sm")
                        po.append(p_)
                        for t in range(NT_N):
                            nc.tensor.matmul(
                                p_[:],
                                vn[:, (h2 * 8 + t) * 65 : (h2 * 8 + t + 1) * 65],
                                ets[t][:, nsl],
                                start=(t == 0),
                                stop=(t == NT_N - 1),
                            )
                        # denominator row -> SBUF (f32r) on ACT, off PE path
                        dn = p_dn.tile([65, 512], F32R, tag="dn")
                        nc.scalar.copy(dn[64:65, :], p_[64:65, :])
                        dns.append(dn)
                    pbs = []
                    for nj in range(2):
                        pb = pp_sm.tile([64, 512], F32, tag="pp_sm")
                        nc.tensor.matmul(
                            pb[:],
                            ones[64:65, 0:64],
                            dns[nj][64:65, :],
                            start=True,
                            stop=True,
                        )
                        pbs.append(pb)
                    for nj in range(2):
                        nsl = slice(nj * 512, (nj + 1) * 512)
                        rcb = p_rcb.tile([64, 512], F32, tag="rcb")
                        nc.vector.reciprocal_approx_fast(rcb[:], pbs[nj][:])
                        if h2 == 0:
                            nc.vector.tensor_mul(
                                OT_hp[0:64, nsl], po[nj][0:64, :], rcb[:]
                            )
                        else:
                            nc.vector.tensor_mul(
                                ot_dst[:, nsl], po[nj][0:64, :], rcb[:]
                            )
                # partition shift 0:64 -> 64:128 via SBUF-to-SBUF DMA
                nc.sync.dma_start(OT_hp[64:128, :], ot_dst[:])

                # incremental proj: acc[i] += OT_hp.T @ wpT[hp-chunk]
                for i in range(NT_N):
                    psj = pp_s.tile([128, 1024], F32, tag="pp_s")
                    for osl in (slice(0, 512), slice(512, 768)):
                        nc.tensor.matmul(
                            psj[:, osl],
                            OT_hp[:, i * 128 : (i + 1) * 128],
                            wpT[:, hp * C + osl.start : hp * C + osl.stop],
                            start=True,
                            stop=True,
                        )
                    nc.vector.tensor_add(acc[i][:], acc[i][:], psj[:, 0:C])

            # ---- final store ----
            for i in range(NT_N):
                nc.sync.dma_start(out[i * 128 : (i + 1) * 128, :], acc[i][:])

    nc.compile()
    return nc


_NC_CACHE = None


def kernel(x, w_qkv, w_proj, b_proj):
    global _NC_CACHE
    if _NC_CACHE is None:
        _NC_CACHE = build_bass()
    nc = _NC_CACHE

    x = np.ascontiguousarray(np.asarray(x, dtype=np.float32))
    w_qkv = np.ascontiguousarray(np.asarray(w_qkv, dtype=np.float32))
    w_proj = np.ascontiguousarray(np.asarray(w_proj, dtype=np.float32))
    b_row = np.ascontiguousarray(
        np.asarray(b_proj, dtype=np.float32).reshape(1, C)
    )

    in_maps = [
        {"x": x[b], "w_qkv": w_qkv, "w_proj": w_proj, "b_proj": b_row}
        for b in range(NCORES)
    ]
    res = run_bass_kernel_spmd(nc, in_maps, list(range(NCORES)))
    return np.stack([res.results[b]["out"] for b in range(NCORES)], axis=0)


# revision 18
# speedup vs baseline: 1.3543x; 1.3543x over previous
"""Trainium2 Bass kernel for multi-head self-attention (B=8, N=1024, C=768, H=12).

Sharding: data-parallel over batch -- one batch element per NeuronCore (8 cores).
Each core computes the full attention for its batch element; no collectives.

Per-core dataflow:
  x [N,C] --PE transpose (f32r single-pass)--> xT [C,N]
  qkvT [3C,N] = w_qkvT.T @ xT            (f32r matmuls, stationary = w chunks)
  per head h:  ST[m,n] = k_h @ q_h^T     (f32r, K=64)
               ET = exp(0.125*ST) -> bf16 (one ACT op per [128,1024] PSUM tile;
                                          no max-subtraction: |scores| < ~3)
               OT_unnorm[d,n], den[n] = [v_h | 1].T @ ET   (bf16 PV, ones-column)
               OT[d,n] = OT_unnorm * bcast(den)^-1  (PE K=1 bcast + fast recip)
  out [N,C] = OT.T @ w_projT + b_proj    (f32r, bias folded in as K=1 matmul)

The PE instruction stream is hand-interleaved so that ST score matmuls (whose
PSUM recycling is gated on the scalar engine's exp throughput) are spaced out
by independent PE work (qkv matmuls, weight transposes, the previous head's
PV accumulation).  Without this the PE micro-stalls every ~1us, the HAM
activity monitor half-clocks the PE, and the whole kernel runs ~2x slower.
"""

import numpy as np

import concourse.bass as bass
import concourse.tile as tile
from concourse import bacc
from concourse import mybir
from concourse.bass_utils import run_bass_kernel_spmd
from concourse.masks import make_identity

N = 1024
C = 768
H = 12
D = 64
O3 = 3 * C  # 2304
NCORES = 8
SCALE = D**-0.5

F32 = mybir.dt.float32
F32R = mybir.dt.float32r
BF16 = mybir.dt.bfloat16
EXP = mybir.ActivationFunctionType.Exp

NT_N = N // 128  # 8 n-blocks
NT_C = C // 128  # 6 c-chunks
NPAIR = H // 2   # 6 head pairs


def build_bass():
    nc = bacc.Bacc("TRN2", target_bir_lowering=False, debug=False, num_devices=NCORES)

    x = nc.dram_tensor("x", [N, C], F32R, kind="ExternalInput").ap()
    w_qkv = nc.dram_tensor("w_qkv", [O3, C], F32R, kind="ExternalInput").ap()
    w_proj = nc.dram_tensor("w_proj", [C, C], F32R, kind="ExternalInput").ap()
    b_proj = nc.dram_tensor("b_proj", [1, C], F32R, kind="ExternalInput").ap()
    out = nc.dram_tensor("out", [N, C], F32, kind="ExternalOutput").ap()

    with tile.TileContext(nc) as tc:
        with (
            tc.tile_pool(name="singles", bufs=1) as singles,
            tc.tile_pool(name="xT", bufs=1) as p_xT,
            tc.tile_pool(name="ld", bufs=4) as p_ld,
            tc.tile_pool(name="wT", bufs=2) as p_wT,
            tc.tile_pool(name="wpT", bufs=1) as p_wpT,
            tc.tile_pool(name="qkvT", bufs=2) as p_qkvT,
            tc.tile_pool(name="et", bufs=16) as p_et,
            tc.tile_pool(name="vn", bufs=1) as p_vn,
            tc.tile_pool(name="OT", bufs=NT_C) as p_OT,
            tc.tile_pool(name="otmp", bufs=2) as p_otmp,
            tc.tile_pool(name="dn", bufs=4) as p_dn,
            tc.tile_pool(name="rcb", bufs=2) as p_rcb,
            tc.tile_pool(name="osb", bufs=2) as p_osb,
            # PSUM: 8 banks total
            tc.tile_pool(name="pp_s", bufs=2, space="PSUM") as pp_s,    # 2x2 banks
            tc.tile_pool(name="pp_sm", bufs=4, space="PSUM") as pp_sm,  # 4x1 bank
        ):
            # ---- setup ----
            identity = singles.tile([128, 128], F32, tag="identity")
            make_identity(nc, identity[:])
            identity_r = singles.tile([128, 128], F32R, tag="identity_r")
            nc.vector.tensor_copy(identity_r[:], identity[:])
            ones_f = singles.tile([128, 128], F32, tag="ones_f")
            nc.vector.memset(ones_f[:], 1.0)
            ones = singles.tile([128, 128], F32R, tag="ones")
            nc.vector.tensor_copy(ones[:], ones_f[:])
            ones_b = singles.tile([128, 1], BF16, tag="ones_b")
            nc.vector.tensor_copy(ones_b[:], ones_f[:, 0:1])
            b_row = singles.tile([1, C], F32R, tag="b_row")
            nc.sync.dma_start(b_row[:], b_proj)

            # persistent vn tile: 16 slots of [v_h block (64) | ones] = 65 cols
            vn = p_vn.tile([128, 16 * 65], BF16, tag="vn")
            ones_cols = bass.AP(
                tensor=vn.tensor, offset=vn.offset + 64, ap=[vn.ap[0], [65, 16], [1, 1]]
            )
            ones_rep = bass.AP(
                tensor=ones_b.tensor,
                offset=ones_b.offset,
                ap=[ones_b.ap[0], [0, 16], [1, 1]],
            )
            nc.vector.tensor_copy(ones_cols, ones_rep)

            def tpose_pair(dst_tensor_ap, dst_off, dst_stride, src, j0, ident, w=128):
                """Two PE transposes into one PSUM tile + one strided DVE copy."""
                pt = pp_sm.tile([128, 2 * w], F32R, tag="pp_sm")
                nc.tensor.transpose(pt[:, 0:w], src(j0), ident)
                nc.tensor.transpose(pt[:, w : 2 * w], src(j0 + 1), ident)
                dst = bass.AP(
                    tensor=dst_tensor_ap.tensor,
                    offset=dst_tensor_ap.offset + dst_off,
                    ap=[dst_tensor_ap.ap[0], [dst_stride, 2], [1, w]],
                )
                nc.vector.tensor_copy(
                    dst, pt[:].rearrange("p (two c) -> p two c", two=2)
                )

            # ---- phase X: x -> xT (single [128, 6*1024] f32r tile) ----
            xT = p_xT.tile([128, NT_C * N], F32R, tag="xT")
            for i in range(NT_N):
                xn = p_ld.tile([128, C], F32R, tag="ld")
                nc.sync.dma_start(xn[:], x[i * 128 : (i + 1) * 128, :])
                for j0 in range(0, NT_C, 2):
                    tpose_pair(
                        xT,
                        j0 * N + i * 128,
                        N,
                        lambda j, xn=xn: xn[:, j * 128 : (j + 1) * 128],
                        j0,
                        identity_r[:],
                    )

            # ---- wproj -> wpT (single [128, 6*768] tile) ----
            wpT = p_wpT.tile([128, NT_C * C], F32R, tag="wpT")
            for i in range(NT_C):
                wpn = p_ld.tile([128, C], F32R, tag="ld")
                nc.sync.dma_start(wpn[:], w_proj[i * 128 : (i + 1) * 128, :])
                for j0 in range(0, NT_C, 2):
                    tpose_pair(
                        wpT,
                        j0 * C + i * 128,
                        C,
                        lambda j, wpn=wpn: wpn[:, j * 128 : (j + 1) * 128],
                        j0,
                        identity_r[:],
                    )

            # ---- emission helpers for the interleaved pair pipeline ----
            OT = [
                p_OT.tile([128, N], F32R, tag="OT", name=f"OT{_}")
                for _ in range(NT_C)
            ]

            def load_wn(hp):
                """Prefetch the 3 natural w_qkv row-blocks for pair hp."""
                wns = []
                for part in range(3):
                    row0 = part * C + hp * 128
                    wn = p_ld.tile([128, C], F32R, tag="ld", name=f"wn{hp}_{part}")
                    nc.sync.dma_start(wn[:], w_qkv[row0 : row0 + 128, :])
                    wns.append(wn)
                return wns

            def emit_wTT_part(wT_t, wns, part):
                """Transpose one w part (3 pt-pairs) into wT tile columns."""
                wn = wns[part]
                for j0 in range(0, NT_C, 2):
                    tpose_pair(
                        wT_t,
                        j0 * 384 + part * 128,
                        384,
                        lambda j, wn=wn: wn[:, j * 128 : (j + 1) * 128],
                        j0,
                        identity_r[:],
                    )

            def emit_qkv_part(blk, wT_t, part):
                ps = pp_s.tile([128, 1024], F32, tag="pp_s")
                for nj in range(2):
                    nsl = slice(nj * 512, (nj + 1) * 512)
                    for j in range(NT_C):
                        nc.tensor.matmul(
                            ps[:, nsl],
                            wT_t[:, j * 384 + part * 128 : j * 384 + (part + 1) * 128],
                            xT[:, j * N + nj * 512 : j * N + nj * 512 + 512],
                            start=(j == 0),
                            stop=(j == NT_C - 1),
                        )
                nc.vector.tensor_copy(blk[:, part * N : (part + 1) * N], ps[:])

            def emit_vnT(blk, h2):
                isl = slice(h2 * 64, h2 * 64 + 64)
                vT = blk[isl, 2 * N : 3 * N]
                for t0 in range(0, NT_N, 2):
                    tpose_pair(
                        vn,
                        (h2 * 8 + t0) * 65,
                        65,
                        lambda t, vT=vT: vT[:, t * 128 : (t + 1) * 128],
                        t0,
                        identity_r[isl, isl],
                        w=64,
                    )

            def emit_st(blk, h2, t, ets):
                rsl = slice(h2 * 64, h2 * 64 + 64)
                qT = blk[rsl, 0:N]
                kT = blk[rsl, N : 2 * N]
                ps = pp_s.tile([128, 1024], F32, tag="pp_s")
                for nj in range(2):
                    nsl = slice(nj * 512, (nj + 1) * 512)
                    nc.tensor.matmul(
                        ps[:, nsl],
                        kT[:, t * 128 : (t + 1) * 128],
                        qT[:, nsl],
                        start=True,
                        stop=True,
                    )
                e = p_et.tile([128, N], BF16, tag="et")
                nc.scalar.activation(e[:], ps[:], EXP, scale=SCALE)
                ets.append(e)

            def emit_pv(h2, nj, ets, po, dns):
                nsl = slice(nj * 512, (nj + 1) * 512)
                p_ = pp_sm.tile([65, 512], F32, tag="pp_sm")
                po.append(p_)
                for t in range(NT_N):
                    nc.tensor.matmul(
                        p_[:],
                        vn[:, (h2 * 8 + t) * 65 : (h2 * 8 + t + 1) * 65],
                        ets[t][:, nsl],
                        start=(t == 0),
                        stop=(t == NT_N - 1),
                    )
                dn = p_dn.tile([65, 512], F32R, tag="dn")
                nc.scalar.copy(dn[64:65, :], p_[64:65, :])
                dns.append(dn)

            def emit_norm(hp, h2, OT_hp, ot_dst, po, dns):
                pbs = []
                for nj in range(2):
                    pb = pp_sm.tile([64, 512], F32, tag="pp_sm")
                    nc.tensor.matmul(
                        pb[:], ones[64:65, 0:64], dns[nj][64:65, :],
                        start=True, stop=True,
                    )
                    pbs.append(pb)
                for nj in range(2):
                    nsl = slice(nj * 512, (nj + 1) * 512)
                    rcb = p_rcb.tile([64, 512], F32, tag="rcb")
                    nc.vector.reciprocal_approx_fast(rcb[:], pbs[nj][:])
                    if h2 == 0:
                        nc.vector.tensor_mul(OT_hp[0:64, nsl], po[nj][0:64, :], rcb[:])
                    else:
                        nc.vector.tensor_mul(ot_dst[:, nsl], po[nj][0:64, :], rcb[:])

            # ---- steady-state pipeline over head pairs ----
            wns = load_wn(0)
            wT_cur = p_wT.tile([128, NT_C * 384], F32R, tag="wT", name="wT0")
            for part in range(3):
                emit_wTT_part(wT_cur, wns, part)

            for hp in range(NPAIR):
                if hp + 1 < NPAIR:
                    wns_next = load_wn(hp + 1)
                    wT_next = p_wT.tile(
                        [128, NT_C * 384], F32R, tag="wT", name=f"wT{hp + 1}"
                    )
                blk = p_qkvT.tile([128, 3 * N], F32R, tag="qkvT")
                OT_hp = OT[hp]
                ot_dst = p_otmp.tile([64, N], F32R, tag="otmp")

                emit_qkv_part(blk, wT_cur, 0)  # q
                emit_qkv_part(blk, wT_cur, 1)  # k

                # ST(h0) stretch, spaced by independent PE filler work
                et0, et1 = [], []
                emit_st(blk, 0, 0, et0)
                emit_qkv_part(blk, wT_cur, 2)  # v
                emit_st(blk, 0, 1, et0)
                for t in (2, 3, 4):
                    if hp + 1 < NPAIR:
                        emit_wTT_part(wT_next, wns_next, t - 2)
                    emit_st(blk, 0, t, et0)
                emit_vnT(blk, 0)
                emit_st(blk, 0, 5, et0)
                emit_vnT(blk, 1)
                emit_st(blk, 0, 6, et0)
                emit_st(blk, 0, 7, et0)

                # ST(h1) stretch, spaced by PV(h0) + its normalization
                po0, dn0 = [], []
                emit_st(blk, 1, 0, et1)
                emit_pv(0, 0, et0, po0, dn0)
                emit_st(blk, 1, 1, et1)
                emit_st(blk, 1, 2, et1)
                emit_pv(0, 1, et0, po0, dn0)
                emit_st(blk, 1, 3, et1)
                emit_st(blk, 1, 4, et1)
                emit_norm(hp, 0, OT_hp, ot_dst, po0, dn0)
                emit_st(blk, 1, 5, et1)
                emit_st(blk, 1, 6, et1)
                emit_st(blk, 1, 7, et1)

                po1, dn1 = [], []
                emit_pv(1, 0, et1, po1, dn1)
                emit_pv(1, 1, et1, po1, dn1)
                emit_norm(hp, 1, OT_hp, ot_dst, po1, dn1)
                # partition shift 0:64 -> 64:128 via SBUF-to-SBUF DMA
                nc.sync.dma_start(OT_hp[64:128, :], ot_dst[:])

                if hp + 1 < NPAIR:
                    wT_cur = wT_next

            # ---- phase C: proj (dense) ----
            for i in range(NT_N):
                ps = pp_s.tile([128, 1024], F32, tag="pp_s")
                for osl in (slice(0, 512), slice(512, 768)):
                    for j in range(NT_C):
                        nc.tensor.matmul(
                            ps[:, osl],
                            OT[j][:, i * 128 : (i + 1) * 128],
                            wpT[:, j * C + osl.start : j * C + osl.stop],
                            start=(j == 0),
                            stop=False,
                        )
                    nc.tensor.matmul(
                        ps[:, osl],
                        ones[0:1, 0:128],
                        b_row[:, osl],
                        start=False,
                        stop=True,
                    )
                osb = p_osb.tile([128, C], F32, tag="osb")
                nc.vector.tensor_copy(osb[:], ps[:, 0:C])
                nc.sync.dma_start(out[i * 128 : (i + 1) * 128, :], osb[:])

    nc.compile()
    return nc


_NC_CACHE = None


def kernel(x, w_qkv, w_proj, b_proj):
    global _NC_CACHE
    if _NC_CACHE is None:
        _NC_CACHE = build_bass()
    nc = _NC_CACHE

    x = np.ascontiguousarray(np.asarray(x, dtype=np.float32))
    w_qkv = np.ascontiguousarray(np.asarray(w_qkv, dtype=np.float32))
    w_proj = np.ascontiguousarray(np.asarray(w_proj, dtype=np.float32))
    b_row = np.ascontiguousarray(
        np.asarray(b_proj, dtype=np.float32).reshape(1, C)
    )

    in_maps = [
        {"x": x[b], "w_qkv": w_qkv, "w_proj": w_proj, "b_proj": b_row}
        for b in range(NCORES)
    ]
    res = run_bass_kernel_spmd(nc, in_maps, list(range(NCORES)))
    return np.stack([res.results[b]["out"] for b in range(NCORES)], axis=0)
